# revision 39
# baseline (speedup 1.0000x reference)
"""Trainium2 Bass kernel for nn_DecGreenNet_product_CP3.

Reference computation:
    lhs  = tanh(input @ Wx1 + bx1) @ Wx2 + bx2          # [B, 512]
    s_i  = sum_n sin(pi*eq*qx_n) * mlp_i(qx_n)           # [8,16] per branch
    rhs  = einsum('bx,dx,fx->bdf', s_a, s_c, s_e)        # [512]
    out  = lhs @ rhs                                     # [B]

Algebraic restructuring used here (validated to ~2e-6 rel err):
    out[b] = tanh(input[b] @ Wx1 + bx1) @ (Wx2 @ rhs) + bx2 @ rhs
    s      = W2^T @ (h1tanh^T @ y) + (sum y) * b2   per quad branch
collapsing the dominant [B,512]x[512,512] GEMM into a matvec.

Sharding: batch B split 8 ways (8192 rows/core); quad nodes split 8 ways
(1024 nodes/core) with a tiny [128,4] AllReduce of the per-core partial
s-vectors (the branch reduction is linear, so partials sum exactly).
"""

import numpy as np

import concourse.bacc as bacc
import concourse.bass as bass
import concourse.mybir as mybir
import concourse.tile as tile
from concourse.bass_utils import run_bass_kernel_spmd

F32 = mybir.dt.float32
F16 = mybir.dt.float16
AF = mybir.ActivationFunctionType
ALU = mybir.AluOpType

NCORES = 8
B, DIN, H = 65536, 3, 512
N, HQ = 8192, 128
S0, RX = 8, 16
BL = B // NCORES          # 8192 batch rows per core
NL = N // NCORES          # 1024 quad nodes per core
NT = NL // 128            # 8 node tiles per branch
CH = 512                  # batch chunk (columns per matmul)
NCH = BL // CH            # 16 chunks
HTILES = H // 128         # 4 h tiles

# scheduling knobs
EMIT_BEFORE = 16          # L1 chunks emitted before the post-collective block
HID_BUFS = 32             # keep all hidden tiles resident

# fp16 scaling: w values are ~1e10-1e11; scale into fp16 range (exact pow2)
RC_SCALE = 2.0 ** -36     # applied to rhs_vec before the fp16 w-matmuls
W_SCALE = 1.0             # applied on psum->sbuf copy of w (total 2^-36)
OUT_SCALE = 2.0 ** 36     # undo in the final output pass

# minimax odd polynomial for sin(t), t in [0, pi]: sin(t)=t*P(t^2), err<2e-5
SIN_C = (0.999984590176674, -0.16663258473611252, 8.312385898666645e-03,
         -1.9316230946716391e-04, 2.1732361127812407e-06)

_CACHED_NC = None

import os
_STAGE = os.environ.get("K_STAGE", "full")  # quad | cc | eins | mainonly | full


def _build():
    nc = bacc.Bacc("TRN2", target_bir_lowering=False, debug=False,
                   num_devices=NCORES)

    xT = nc.dram_tensor("xT", [DIN + 1, BL], F16, kind="ExternalInput").ap()
    wx1a = nc.dram_tensor("wx1a", [DIN + 1, H], F16, kind="ExternalInput").ap()
    wx2t = nc.dram_tensor("wx2t", [H, H], F16, kind="ExternalInput").ap()
    bx2r = nc.dram_tensor("bx2r", [128, 64], F32, kind="ExternalInput").ap()
    qxa = nc.dram_tensor("qxa", [6, NL], F16, kind="ExternalInput").ap()
    qxc = nc.dram_tensor("qxc", [128, 3 * NT], F32, kind="ExternalInput").ap()
    wqa = nc.dram_tensor("wqa", [6, HQ], F16, kind="ExternalInput").ap()
    wq2 = nc.dram_tensor("wq2", [HQ, 3 * HQ], F32, kind="ExternalInput").ap()
    bq2r = nc.dram_tensor("bq2r", [3, HQ], F32, kind="ExternalInput").ap()
    eqb = nc.dram_tensor("eqb", [128, 1], F32, kind="ExternalInput").ap()
    out_d = nc.dram_tensor("out", [BL], F32, kind="ExternalOutput").ap()

    global _APS
    _APS = (xT, wx1a, wx2t, bx2r, qxa, qxc, wqa, wq2, bq2r, eqb, out_d)
    with tile.TileContext(nc) as tc:
        _body(nc, tc)
    nc.compile()
    return nc


def _body(nc, tc):
        xT, wx1a, wx2t, bx2r, qxa, qxc, wqa, wq2, bq2r, eqb, out_d = _APS
        with (
            tc.tile_pool(name="const", bufs=1) as constp,
            tc.tile_pool(name="qsb", bufs=1) as qsb,
            tc.tile_pool(name="h1p", bufs=4) as h1p,
            tc.tile_pool(name="dram", bufs=2, space="DRAM") as dram,
            tc.tile_pool(name="tinyp", bufs=1, space="PSUM") as tinyp,
            tc.tile_pool(name="mainsb", bufs=1) as mainsb,
            tc.tile_pool(name="orowp", bufs=3) as orowp,
            tc.tile_pool(name="esb", bufs=2) as esb,
            tc.tile_pool(name="hidp", bufs=HID_BUFS) as hidp,
            tc.tile_pool(name="prep", bufs=2, space="PSUM") as prep,
            tc.tile_pool(name="outp", bufs=2, space="PSUM") as outpp,
        ):
            ones128 = constp.tile([128, 1], F32)
            nc.vector.memset(ones128, 1.0)

            # ---------------- quad phase DMAs ----------------
            # per-branch tiles so every matmul operand starts at partition 0
            # y-polynomial inputs first (critical path to the collective)
            qxc_sb = qsb.tile([128, 3 * NT], F32, tag="qxc")
            nc.sync.dma_start(out=qxc_sb, in_=qxc)
            eqb_sb = qsb.tile([128, 1], F32, tag="eqb")
            nc.sync.dma_start(out=eqb_sb, in_=eqb)
            qxa_sb, wqa_sb, bq2r_sb = [], [], []
            qeng = [nc.gpsimd, nc.sync, nc.gpsimd]
            for br in range(3):
                e = qeng[br]
                t = qsb.tile([2, NL], F16, tag=f"qxa{br}")
                e.dma_start(out=t, in_=qxa[2 * br:2 * br + 2, :])
                qxa_sb.append(t)
                t = qsb.tile([2, HQ], F16, tag=f"wqa{br}")
                e.dma_start(out=t, in_=wqa[2 * br:2 * br + 2, :])
                wqa_sb.append(t)
                t = qsb.tile([1, HQ], F32, tag=f"bq2r{br}")
                e.dma_start(out=t, in_=bq2r[br:br + 1, :])
                bq2r_sb.append(t)
            wq2_sb = qsb.tile([HQ, 3 * HQ], F32, tag="wq2")
            nc.sync.dma_start(out=wq2_sb, in_=wq2)

            qcut = int(os.environ.get("K_QCUT", "99"))

            def qdump(ap2d):
                p, c = ap2d.shape[0], ap2d.shape[1]
                nc.sync.dma_start(
                    out=out_d[0:p * c].rearrange("(p c) -> p c", c=c),
                    in_=ap2d)

            if qcut <= 1:
                qdump(qxc_sb[:, 0:3])
                return

            # y = sin(pi*eq*qx) via odd minimax polynomial on the DVE
            # (keeps ScalarE on a single act-table set: Tanh only)
            eqpi = qsb.tile([128, 1], F32, tag="eqpi")
            nc.vector.tensor_scalar_mul(eqpi, eqb_sb, float(np.pi))
            tq = qsb.tile([128, 3 * NT], F32, tag="tq")
            nc.vector.tensor_scalar_mul(tq, qxc_sb, eqpi[:, 0:1])
            t2 = qsb.tile([128, 3 * NT], F32, tag="t2")
            nc.vector.tensor_tensor(out=t2, in0=tq, in1=tq, op=ALU.mult)
            pp = qsb.tile([128, 3 * NT], F32, tag="pp")
            c1, c3, c5, c7, c9 = [float(v) for v in SIN_C]
            nc.vector.tensor_scalar(out=pp, in0=t2, scalar1=c9, scalar2=c7,
                                    op0=ALU.mult, op1=ALU.add)
            for cof in (c5, c3, c1):
                nc.vector.tensor_tensor(out=pp, in0=pp, in1=t2, op=ALU.mult)
                nc.vector.tensor_scalar_add(pp, pp, cof)
            y_sb = qsb.tile([128, 3 * NT], F16, tag="ysb")
            nc.vector.tensor_tensor(out=y_sb, in0=pp, in1=tq, op=ALU.mult)
            if qcut <= 2:
                qdump(y_sb[:, 0:3])
                return

            # ---------------- quad branches ----------------
            # qsmall columns: 0-2 = z per branch, 3-5 = sy per branch (row 0),
            # 6-8 = s per branch
            qsmall = tinyp.tile([128, 12], F32, tag="tiny")
            nc.vector.memset(qsmall[:, 3:6], 0.0)
            for br in range(3):
                h1s = []
                for half in range(2):
                    qpre = prep.tile([128, 512], F32, tag="pre")
                    for i2 in range(4):
                        i = half * 4 + i2
                        nc.tensor.matmul(
                            qpre[:, i2 * 128:(i2 + 1) * 128],
                            lhsT=qxa_sb[br][:, i * 128:(i + 1) * 128],
                            rhs=wqa_sb[br],
                            start=True, stop=True)
                    h1 = h1p.tile([128, 512], F16, tag="h1")
                    nc.scalar.activation(out=h1, in_=qpre, func=AF.Tanh)
                    h1s.append(h1)
                # z[h] = sum_n h1[n,h]*y[n], accumulated over 8 node tiles
                for i in range(NT):
                    nc.tensor.matmul(
                        qsmall[:, br:br + 1],
                        lhsT=h1s[i // 4][:, (i % 4) * 128:(i % 4 + 1) * 128],
                        rhs=y_sb[:, br * NT + i:br * NT + i + 1],
                        start=(i == 0), stop=(i == NT - 1))
                if qcut <= 5:
                    continue
                # sy = sum_n y[n]  -> row 0 of column 3+br
                ysum = qsb.tile([128, 1], F32, tag="ysum")
                nc.vector.tensor_reduce(
                    out=ysum, in_=y_sb[:, br * NT:(br + 1) * NT],
                    axis=mybir.AxisListType.X, op=ALU.add)
                nc.tensor.matmul(
                    qsmall[0:1, 3 + br:4 + br], lhsT=ysum[:, 0:1],
                    rhs=ones128[:, 0:1], start=True, stop=True)

            if qcut <= 3:
                qdump(h1s[0][:, 0:3])
                return

            z_sb = qsb.tile([128, 6], F32, tag="zsb")
            if qcut <= 5:
                nc.vector.tensor_copy(out=z_sb[:, 0:3], in_=qsmall[:, 0:3])
                qdump(z_sb[:, 0:3])
                return
            nc.vector.tensor_copy(out=z_sb, in_=qsmall[:, 0:6])
            if qcut <= 6:
                qdump(z_sb[:, 0:6])
                return
            # s = W2^T z + sy * b2 per branch -> columns 6..8
            for br in range(3):
                nc.tensor.matmul(
                    qsmall[:, 6 + br:7 + br],
                    lhsT=wq2_sb[:, br * HQ:(br + 1) * HQ],
                    rhs=z_sb[:, br:br + 1], start=True, stop=False)
                nc.tensor.matmul(
                    qsmall[:, 6 + br:7 + br],
                    lhsT=bq2r_sb[br],
                    rhs=z_sb[0:1, 3 + br:4 + br], start=False, stop=True)
            s_sb = qsb.tile([128, 3], F32, tag="ssb")
            nc.vector.tensor_copy(out=s_sb, in_=qsmall[:, 6:9])

            if _STAGE == "quad":
                nc.sync.dma_start(out=out_d[0:384],
                                  in_=s_sb.rearrange("p c -> (p c)"))
                return

            # ---------------- AllReduce of partial s ----------------
            cc_in = dram.tile([128, 3], F32, tag="ccin")
            cc_out = dram.tile([128, 3], F32, tag="ccout")
            nc.gpsimd.dma_start(out=cc_in, in_=s_sb)
            nc.gpsimd.collective_compute(
                "AllReduce", ALU.add,
                replica_groups=[list(range(NCORES))],
                ins=[cc_in[:].opt()], outs=[cc_out[:].opt()])
            if _STAGE == "cc":
                sg_sb = qsb.tile([128, 3], F32, tag="sgsb")
                nc.gpsimd.dma_start(out=sg_sb, in_=cc_out)
                qdump(sg_sb)
                return

            # ---------------- main phase DMAs ----------------
            xT_sb = mainsb.tile([DIN + 1, BL], F16, tag="xT")
            nc.sync.dma_start(out=xT_sb, in_=xT)
            wx1a_sb = mainsb.tile([DIN + 1, H], F16, tag="wx1a")
            nc.sync.dma_start(out=wx1a_sb, in_=wx1a)
            wx2t_sb = mainsb.tile([128, 4 * H], F16, tag="wx2t")
            nc.sync.dma_start(out=wx2t_sb,
                              in_=wx2t.rearrange("(jt p) i -> p jt i", p=128))
            bx2r_sb = mainsb.tile([128, 64], F32, tag="bx2r")
            nc.sync.dma_start(out=bx2r_sb, in_=bx2r)

            # ---------------- main L1 chunks (emitter) ----------------
            hid_tiles = {}

            def emit_l1(c):
                tiles = []
                for half in range(2):
                    pre = prep.tile([128, 1024], F32, tag="pre")
                    for k in range(2):
                        ht = half * 2 + k
                        nc.tensor.matmul(
                            pre[:, k * 512:(k + 1) * 512],
                            lhsT=wx1a_sb[:, ht * 128:(ht + 1) * 128],
                            rhs=xT_sb[:, c * CH:(c + 1) * CH],
                            start=True, stop=True)
                    hid = hidp.tile([128, 1024], F16, tag="hid")
                    nc.scalar.activation(out=hid, in_=pre, func=AF.Tanh)
                    tiles.append(hid)
                hid_tiles[c] = tiles

            for c in range(EMIT_BEFORE):
                emit_l1(c)

            # ---------------- post-collective small compute ----------------
            # s columns [128]=(b*16+x) -> sT[16x, (br,8b)] straight from the
            # collective's DRAM output (single strided DMA)
            sT_sb = esb.tile([16, 24], F32, tag="sT")
            nc.sync.dma_start(
                out=sT_sb,
                in_=cc_out.rearrange("(b x) c -> x c b", x=16))
            # E[x, d*8+f] = s_c[d,x] * s_e[f,x]
            sc_ap = sT_sb[:, 8:16]
            se_ap = sT_sb[:, 16:24]
            in0 = bass.AP(tensor=sc_ap.tensor, offset=sc_ap.offset,
                          ap=[sc_ap.ap[0], sc_ap.ap[1], [0, 8]])
            in1 = bass.AP(tensor=se_ap.tensor, offset=se_ap.offset,
                          ap=[se_ap.ap[0], [0, 8], se_ap.ap[1]])
            E_sb = esb.tile([16, 64], F32, tag="E")
            nc.vector.tensor_tensor(
                out=E_sb.rearrange("p (d f) -> p d f", f=8),
                in0=in0, in1=in1, op=ALU.mult)
            # rhs_vec[b, d*8+f] = sum_x s_a[b? -> see below] ;
            # out[b,df] = sum_x sT_a[x,b] * E[x,df]
            rhsp = tinyp.tile([64, 8], F32, tag="tiny")
            nc.tensor.matmul(rhsp, lhsT=E_sb, rhs=sT_sb[:, 0:8],
                             start=True, stop=True)
            r_sb = esb.tile([64, 8], F32, tag="rsb")
            nc.vector.tensor_copy(out=r_sb, in_=rhsp)
            # [64 df, 8 b] -> [128 (b%2*64+df), 4 (b//2)]: two strided DMAs
            rc_sb = esb.tile([128, 4], F32, tag="rc")
            nc.sync.dma_start(
                out=rc_sb[0:64, :],
                in_=r_sb.rearrange("pl (t ph) -> pl t ph", ph=2)[:, :, 0])
            nc.sync.dma_start(
                out=rc_sb[64:128, :],
                in_=r_sb.rearrange("pl (t ph) -> pl t ph", ph=2)[:, :, 1])
            if _STAGE == "eins":
                qdump(rc_sb)
                return
            if _STAGE == "eins2":
                qdump(r_sb)
                return
            rc16 = esb.tile([128, 4], F16, tag="rc16")
            nc.vector.tensor_scalar_mul(rc16, rc_sb, float(RC_SCALE))
            # w = Wx2 @ rhs_vec  as [128, 4] (h = it*128 + p), fp16-scaled
            wps = tinyp.tile([128, 4], F32, tag="tiny")
            for it in range(4):
                for jt in range(4):
                    nc.tensor.matmul(
                        wps[:, it:it + 1],
                        lhsT=wx2t_sb[:, jt * H + it * 128:jt * H + (it + 1) * 128],
                        rhs=rc16[:, jt:jt + 1],
                        start=(jt == 0), stop=(jt == 3))
            w_sb = esb.tile([128, 4], F16, tag="wsb")
            nc.vector.tensor_scalar_mul(w_sb, wps, float(W_SCALE))
            # c (scalar) replicated over 16 partitions
            c16p = tinyp.tile([16, 1], F32, tag="tiny")
            for jt in range(4):
                nc.tensor.matmul(
                    c16p, lhsT=bx2r_sb[:, jt * 16:(jt + 1) * 16],
                    rhs=rc_sb[:, jt:jt + 1],
                    start=(jt == 0), stop=(jt == 3))
            c16_sb = esb.tile([16, 1], F32, tag="c16")
            nc.vector.tensor_copy(out=c16_sb, in_=c16p)

            # ---------------- rest of L1 + dots ----------------
            def emit_dot(c):
                op = outpp.tile([1, 512], F32, tag="outp")
                for ht in range(HTILES):
                    nc.tensor.matmul(
                        op,
                        lhsT=w_sb[:, ht:ht + 1],
                        rhs=hid_tiles[c][ht // 2][:, (ht % 2) * 512:(ht % 2 + 1) * 512],
                        start=(ht == 0), stop=(ht == HTILES - 1))
                orow = orowp.tile([1, 512], F32, tag="outrow")
                nc.vector.tensor_scalar(
                    out=orow, in0=op, scalar1=float(OUT_SCALE),
                    scalar2=c16_sb[0:1, 0:1], op0=ALU.mult, op1=ALU.add)
                nc.sync.dma_start(
                    out=out_d[c * CH:(c + 1) * CH].rearrange("(o b) -> o b", o=1),
                    in_=orow)

            for c in range(EMIT_BEFORE, NCH):
                emit_l1(c)
            for c in range(NCH):
                emit_dot(c)


def _get_nc():
    global _CACHED_NC
    if _CACHED_NC is None:
        _CACHED_NC = _build()
    return _CACHED_NC


def _prep_in_maps(inputs):
    f = lambda k: np.ascontiguousarray(np.asarray(inputs[k], np.float32))
    inputx = f("input")
    eq = float(np.asarray(inputs["eq_param"]).reshape(-1)[0])
    Wx1, bx1 = f("Wx1"), f("bx1")
    Wx2, bx2 = f("Wx2"), f("bx2")

    wx1a = np.concatenate([Wx1, bx1[None, :]], axis=0).astype(np.float16)
    wx2t = np.ascontiguousarray(Wx2.T).astype(np.float16)       # [512, 512]
    bx2c = np.ascontiguousarray(bx2.reshape(4, 128).T)          # [128, 4]
    bx2r = np.repeat(bx2c, 16, axis=1)                          # [128, 64]
    wqa = np.empty((6, HQ), np.float16)
    bq2r = np.empty((3, HQ), np.float32)
    wq2 = np.empty((HQ, 3 * HQ), np.float32)
    qs = []
    for br, (qk, w1k, b1k, w2k, b2k) in enumerate([
            ("quad_x0", "Wq01", "bq01", "Wq02", "bq02"),
            ("quad_x1", "Wq11", "bq11", "Wq12", "bq12"),
            ("quad_x2", "Wq21", "bq21", "Wq22", "bq22")]):
        wqa[2 * br] = f(w1k)[0]
        wqa[2 * br + 1] = f(b1k)
        wq2[:, br * HQ:(br + 1) * HQ] = f(w2k)
        bq2r[br] = f(b2k)
        qs.append(f(qk)[:, 0])
    eqb = np.full((128, 1), eq, np.float32)

    shared = dict(wx1a=wx1a, wx2t=wx2t, bx2r=bx2r, wqa=wqa, wq2=wq2,
                  bq2r=bq2r, eqb=eqb)
    in_maps = []
    ones_row = np.ones((1, BL), np.float32)
    for c in range(NCORES):
        ish = inputx[c * BL:(c + 1) * BL]                        # [8192, 3]
        xTm = np.concatenate([ish.T, ones_row], axis=0)          # [4, 8192]
        qxa = np.empty((6, NL), np.float32)
        qxc = np.empty((128, 3 * NT), np.float32)
        for br in range(3):
            sh = qs[br][c * NL:(c + 1) * NL]
            qxa[2 * br] = sh
            qxa[2 * br + 1] = 1.0
            qxc[:, br * NT:(br + 1) * NT] = sh.reshape(NT, 128).T
        m = dict(shared)
        m["xT"] = np.ascontiguousarray(xTm).astype(np.float16)
        m["qxa"] = qxa.astype(np.float16)
        m["qxc"] = np.ascontiguousarray(qxc)
        in_maps.append(m)
    return in_maps


def _run(inputs, **kw):
    nc = _get_nc()
    in_maps = _prep_in_maps(inputs)
    res = run_bass_kernel_spmd(nc, in_maps, list(range(NCORES)), **kw)
    out = np.concatenate([res.results[c]["out"].reshape(-1)
                          for c in range(NCORES)]).astype(np.float32)
    return out, res


def kernel(**inputs) -> np.ndarray:
    out, _ = _run(inputs)
    return out


def kernel_traced(**inputs):
    """Correctness + NTFF profile (exec_time_ns) in one run."""
    return _run(inputs, trace=True)
